# revision 41
# baseline (speedup 1.0000x reference)
"""Trainium2 Bass kernel for a 64-expert top-8 SwiGLU MoE layer.

Contract: kernel(**inputs) takes the FULL unsharded inputs
  hidden_states [2, 2048, 1024] f32, gate_w [64, 1024] f32,
  w_gate [64, 768, 1024] f32, w_up [64, 768, 1024] f32,
  w_down [64, 1024, 768] f32
and returns the full [2, 2048, 1024] f32 output.

Sharding: expert-parallel over 8 NeuronCores — core c owns experts
[8c, 8c+8). Every core computes the fp32 gate over all 4096 tokens
(top-8 of 64, renormalized), packs (token_id + weight) into one fp32
composite per (token, local expert), transposes those via the PE into
per-expert streams, compacts each stream with gpsimd sparse_gather
(sentinel-padded so the tail needs no fixup), then runs the SwiGLU
expert FFNs in bf16 (fp32 accumulation) on dma_gathered token rows and
scatter-adds the weighted bf16 rows into its partial output. The host
sums the 8 per-core partials.

gpsimd library hygiene: the only Pool ops are sparse_gather (lib 8)
and dma_gather/dma_scatter_add (mlp lib), grouped so exactly two
library loads occur, one of which hides under the gate phase.
"""

import sys

for _p in ("/opt/trn_rl_repo",):
    if _p not in sys.path:
        sys.path.insert(0, _p)

import numpy as np
import ml_dtypes

import concourse.bass as bass  # noqa: F401  (registers engine classes)
import concourse.bacc as bacc
import concourse.mybir as mybir
import concourse.tile as tile
from concourse import bass_utils

AF = mybir.ActivationFunctionType
ALU = mybir.AluOpType
DT = mybir.dt
BF16 = ml_dtypes.bfloat16

N_CORES = 8
N_LOC = 8          # experts per core
T = 4096           # tokens
TPAD = 16          # dummy rows for the sentinel token id T
D = 1024
F = 768
E = 64
CAP = 640          # per-expert token capacity (multiple of 128)
KD = D // 128      # 8
KF = F // 128      # 6
CT = CAP // 128    # 5
N_TT = T // 128    # 32 token tiles
SENT = float(T)    # sentinel composite value: token T, weight 0


def _build_nc(n_devices=N_CORES, repeat=1):
    C = CAP
    VE_F = (T + C) // 16       # 296 sparse_gather free size per expert
    n_chunks = [(0, 512), (512, C - 512)]
    d_chunks = [(0, 512), (512, 512)]

    nc = bacc.Bacc("TRN2", target_bir_lowering=False, debug=False,
                   num_devices=n_devices)

    xT = nc.dram_tensor("xT", [D, T], DT.float32, kind="ExternalInput")
    xb = nc.dram_tensor("xb", [T + TPAD, D], DT.bfloat16,
                        kind="ExternalInput")
    gwT = nc.dram_tensor("gwT", [D, E], DT.float32, kind="ExternalInput")
    tok1 = nc.dram_tensor("tok1", [128, N_TT], DT.float32,
                          kind="ExternalInput")
    ident = nc.dram_tensor("ident", [128, 128], DT.float32,
                           kind="ExternalInput")
    iota16 = nc.dram_tensor("iota16", [128, 8], DT.int16,
                            kind="ExternalInput")
    wgT = nc.dram_tensor("wgT", [N_LOC, D, F], DT.bfloat16,
                         kind="ExternalInput")
    wuT = nc.dram_tensor("wuT", [N_LOC, D, F], DT.bfloat16,
                         kind="ExternalInput")
    wdT = nc.dram_tensor("wdT", [N_LOC, F, D], DT.bfloat16,
                         kind="ExternalInput")
    y = nc.dram_tensor("y", [T + TPAD, D], DT.bfloat16,
                       kind="ExternalOutput")
    vt_dram = nc.dram_tensor("vt_scratch", [N_LOC, T], DT.float32,
                             kind="Internal")
    ti_dram = nc.dram_tensor("ti_scratch", [N_LOC, 16, 128], DT.int16,
                             kind="Internal")
    wf_dram = nc.dram_tensor("wf_scratch", [N_LOC, 16, CAP // 16],
                             DT.float32, kind="Internal")

    with tile.TileContext(nc) as tc:
      for _rep in range(repeat):  # >1 only for wall-clock benchmarking
        with (
            tc.tile_pool(name="gconst", bufs=1) as gconst,
            tc.tile_pool(name="vtp", bufs=1) as vtp,
            tc.tile_pool(name="route", bufs=1) as route,
            tc.tile_pool(name="decp", bufs=1) as decp,
            tc.tile_pool(name="wrp", bufs=1) as wrp,
        ):
            # ---- phase A: fp32 gate + composite pack + PE transpose ----
            with (
                tc.tile_pool(name="gx", bufs=2) as gx,
                tc.tile_pool(name="gps", bufs=3, space="PSUM") as gps,
                tc.tile_pool(name="tps", bufs=2, space="PSUM") as tps,
                tc.tile_pool(name="gtmp", bufs=3) as gtmp,
            ):
                # first (small) token chunk before the constant loads — SP
                # issues DMAs in order and the matmuls need tokens first;
                # a small chunk 0 cuts the PE startup wait
                chunk_lims = [0, 256, 1024, 2048, 3072, 4096]
                xt_ch = gx.tile([128, KD, 256], DT.float32, tag="xt")
                nc.sync.dma_start(
                    xt_ch[:], xT.ap()[:, 0:256].rearrange(
                        "(kc p) t -> p kc t", p=128))

                gw_sb = gconst.tile([128, KD, E], DT.float32)
                nc.sync.dma_start(gw_sb[:], gwT.ap().rearrange(
                    "(kc p) e -> p kc e", p=128))
                id_sb = gconst.tile([128, 128], DT.float32)
                nc.sync.dma_start(id_sb[:], ident.ap())
                tok1_sb = gconst.tile([128, N_TT], DT.float32)
                nc.sync.dma_start(tok1_sb[:], tok1.ap())
                io16_sb = gconst.tile([128, 8], DT.int16)
                nc.sync.dma_start(io16_sb[:], iota16.ap())

                # vT[e, t] = token0(t) + weight(t, e) if e in top8(t),
                # else -1, for the 8 local experts.
                vT = vtp.tile([N_LOC, T], DT.float32)

                for ch in range(len(chunk_lims) - 1):
                    c0, c1 = chunk_lims[ch], chunk_lims[ch + 1]
                    if ch > 0:
                        xt_ch = gx.tile([128, KD, c1 - c0], DT.float32,
                                        tag="xt")
                        nc.sync.dma_start(
                            xt_ch[:],
                            xT.ap()[:, c0:c1].rearrange(
                                "(kc p) t -> p kc t", p=128))
                    for ti in range((c1 - c0) // 128):
                        tt = c0 // 128 + ti
                        psL = gps.tile([128, E], DT.float32, tag="psL")
                        for kc in range(KD):
                            nc.tensor.matmul(
                                psL[:],
                                xt_ch[:, kc, ti * 128:(ti + 1) * 128],
                                gw_sb[:, kc, :],
                                start=(kc == 0), stop=(kc == KD - 1),
                            )
                        mx8 = gtmp.tile([128, 8], DT.float32, tag="mx8")
                        nc.vector.max(mx8[:], psL[:])
                        negm = gtmp.tile([128, 1], DT.float32, tag="negm")
                        nc.vector.tensor_scalar_mul(negm[:], mx8[:, 0:1],
                                                    -1.0)
                        e8 = gtmp.tile([128, 8], DT.float32, tag="e8")
                        s8 = gtmp.tile([128, 1], DT.float32, tag="s8")
                        nc.scalar.activation(e8[:], mx8[:], AF.Exp,
                                             bias=negm[:], accum_out=s8[:])
                        rcp = gtmp.tile([128, 1], DT.float32, tag="rcp")
                        nc.vector.reciprocal(rcp[:], s8[:])
                        # local experts live in columns 0..8
                        ea8 = gtmp.tile([128, 8], DT.float32, tag="ea8")
                        nc.scalar.activation(ea8[:], psL[:, 0:N_LOC], AF.Exp,
                                             bias=negm[:])
                        ge8 = gtmp.tile([128, 8], DT.float32, tag="ge8")
                        nc.vector.tensor_scalar(ge8[:], psL[:, 0:N_LOC],
                                                mx8[:, 7:8], None,
                                                op0=ALU.is_ge)
                        wm8 = gtmp.tile([128, 8], DT.float32, tag="wm8")
                        nc.vector.scalar_tensor_tensor(
                            wm8[:], ea8[:], rcp[:], ge8[:],
                            op0=ALU.mult, op1=ALU.mult)
                        # composite: (wm8 + tok1)*ge - 1 =
                        #   tok0 + w (selected) / -1 (not selected)
                        vp = gtmp.tile([128, 8], DT.float32, tag="vp")
                        nc.vector.scalar_tensor_tensor(
                            vp[:], wm8[:], tok1_sb[:, tt:tt + 1], ge8[:],
                            op0=ALU.add, op1=ALU.mult)
                        vpack = gtmp.tile([128, 8], DT.float32, tag="vpack")
                        nc.vector.tensor_scalar_add(vpack[:], vp[:], -1.0)
                        psT = tps.tile([N_LOC, 128], DT.float32, tag="psT")
                        nc.tensor.transpose(psT[:], vpack[:], id_sb[:])
                        nc.vector.tensor_copy(
                            vT[:, tt * 128:(tt + 1) * 128], psT[:])
                    # stream vT to DRAM per chunk so only the last write
                    # sits on the routing critical path
                    nc.sync.dma_start(vt_dram.ap()[:, c0:c1],
                                      vT[:, c0:c1])

            # ---- phase A2: per-expert compaction (sentinel-padded) ----
            # ve[e] = [16, 296]: head = vT row e partition-major, tail =
            # 640 sentinels. After compaction the first 640 outputs are
            # real-tokens-then-sentinels regardless of tail behaviour.
            ve_all = route.tile([16, N_LOC, VE_F], DT.float32)
            nc.sync.dma_start(
                ve_all[:, :, 0:T // 16],
                vt_dram.ap().rearrange("e (p c) -> p e c", p=16))
            nc.vector.memset(ve_all[:, :, T // 16:], SENT)

            toks = []
            nfs = []
            for e in range(N_LOC):
                tk = route.tile([16, VE_F], DT.float32, tag=f"tk{e}")
                nf = route.tile([1, 1], DT.uint32, tag=f"nf{e}")
                nc.gpsimd.sparse_gather(tk[:], ve_all[:, e, :],
                                        num_found=nf[:])
                toks.append(tk)
                nfs.append(nf)

            # decode composites: token ids (int16) and fractional gate
            # weights, bounced through DRAM so the index replication can be
            # a Pool dma_gather (row-replicate) and the weight unwrap an
            # arbitrary-rearrange DRAM read.
            ti_all = decp.tile([16, N_LOC, 128], DT.int16)
            tf_all = decp.tile([16, N_LOC, C // 16], DT.float32)
            wf_all = decp.tile([16, N_LOC, C // 16], DT.float32)
            wrow_all = wrp.tile([128, N_LOC, CT], DT.float32)
            nc.vector.memset(ti_all[:, :, C // 16:], 0)
            for e in range(N_LOC):
                tv = toks[e][:, 0:C // 16]
                # f32 -> i16 cast truncates toward zero = floor (tv >= 0)
                nc.vector.tensor_copy(ti_all[:, e, 0:C // 16], tv)
                nc.vector.tensor_copy(tf_all[:, e, :], ti_all[:, e, 0:C // 16])
                nc.vector.tensor_tensor(wf_all[:, e, :], tv, tf_all[:, e, :],
                                        op=ALU.subtract)
                nc.sync.dma_start(ti_dram.ap()[e], ti_all[:, e, :])
                nc.sync.dma_start(wf_dram.ap()[e], wf_all[:, e, :])
                # wrow[g*16+p, st] = wf[p, st*8+g]
                nc.sync.dma_start(
                    wrow_all[:, e, :],
                    wf_dram.ap()[e].rearrange("p (st g) -> g p st", g=8))
            wrows = [wrow_all[:, e, :] for e in range(N_LOC)]

            # ---- phase B: expert FFNs (bf16, fp32 accum) ----
            with (
                tc.tile_pool(name="xg", bufs=2) as xg,
                tc.tile_pool(name="rip", bufs=2) as rip,
                tc.tile_pool(name="wsb", bufs=2) as wsb,
                tc.tile_pool(name="hsb", bufs=2) as hsb,
                tc.tile_pool(name="ysb", bufs=2) as ysbp,
                tc.tile_pool(name="bps", bufs=2, space="PSUM") as bps,
                tc.tile_pool(name="dps", bufs=2, space="PSUM") as dps,
            ):
                ris = [None] * N_LOC
                XTs = [None] * N_LOC

                def prefetch(e):
                    # replicate idx row [16,128] -> [128,128] on Pool, then
                    # transpose-gather this expert's token rows
                    ri = rip.tile([128, 1, 128], DT.int16, tag="ri")
                    nc.gpsimd.dma_gather(ri[:], ti_dram.ap()[e],
                                         io16_sb[:], 128, 128, 128)
                    XT = xg.tile([128, KD, C], DT.bfloat16, tag="XT")
                    nc.gpsimd.dma_gather(XT[:], xb.ap(), ri[:, 0, 0:C // 16],
                                         C, C, D, transpose=True)
                    ris[e] = ri
                    XTs[e] = XT

                prefetch(0)
                for e in range(N_LOC):
                    XT = XTs[e]
                    # weight loads go on the ACT/DVE DGE queues so the
                    # routing-critical DMAs (vt/ve/decode) own the SP queue
                    wg_sb = wsb.tile([128, KD, F], DT.bfloat16, tag="wg")
                    nc.scalar.dma_start(wg_sb[:], wgT.ap()[e].rearrange(
                        "(kc p) f -> p kc f", p=128))
                    wu_sb = wsb.tile([128, KD, F], DT.bfloat16, tag="wu")
                    nc.scalar.dma_start(wu_sb[:], wuT.ap()[e].rearrange(
                        "(kc p) f -> p kc f", p=128))
                    wd_sb = wsb.tile([128, KF, D], DT.bfloat16, tag="wd")
                    nc.scalar.dma_start(wd_sb[:], wdT.ap()[e].rearrange(
                        "(kf p) d -> p kf d", p=128))

                    ht = hsb.tile([128, KF, C], DT.bfloat16, tag="ht")
                    ysb = ysbp.tile([128, CT, D], DT.bfloat16, tag="ysb")

                    def gu_chunk(c0, cn):
                        for ft in range(KF):
                            psg = bps.tile([128, 512], DT.float32,
                                           tag="psg")
                            psu = bps.tile([128, 512], DT.float32,
                                           tag="psu")
                            for kc in range(KD):
                                nc.tensor.matmul(
                                    psg[:, :cn],
                                    wg_sb[:, kc, ft * 128:(ft + 1) * 128],
                                    XT[:, kc, c0:c0 + cn],
                                    start=(kc == 0), stop=(kc == KD - 1))
                            for kc in range(KD):
                                nc.tensor.matmul(
                                    psu[:, :cn],
                                    wu_sb[:, kc, ft * 128:(ft + 1) * 128],
                                    XT[:, kc, c0:c0 + cn],
                                    start=(kc == 0), stop=(kc == KD - 1))
                            sg = hsb.tile([128, 512], DT.bfloat16, tag="sg")
                            nc.scalar.activation(sg[:, :cn], psg[:, :cn],
                                                 AF.Sigmoid)
                            t1 = hsb.tile([128, 512], DT.bfloat16, tag="t1")
                            nc.vector.tensor_tensor(
                                t1[:, :cn], sg[:, :cn],
                                psu[:, :cn], op=ALU.mult)
                            nc.vector.tensor_tensor(
                                ht[:, ft, c0:c0 + cn], t1[:, :cn],
                                psg[:, :cn], op=ALU.mult)

                    def down_tile(st):
                        for (d0, dn) in d_chunks:
                            psd = dps.tile([128, 512], DT.float32,
                                           tag="psd")
                            for kf in range(KF):
                                nc.tensor.matmul(
                                    psd[:, :dn],
                                    ht[:, kf, st * 128:(st + 1) * 128],
                                    wd_sb[:, kf, d0:d0 + dn],
                                    start=(kf == 0), stop=(kf == KF - 1))
                            nc.vector.tensor_scalar(
                                ysb[:, st, d0:d0 + dn], psd[:, :dn],
                                wrows[e][:, st:st + 1], None,
                                op0=ALU.mult)

                    gu_chunk(0, 512)
                    for st in range(4):
                        down_tile(st)

                    if e + 1 < N_LOC:
                        prefetch(e + 1)

                    # slots 512..639 are all sentinels when this expert got
                    # <= 512 tokens (num_found = real + 640 sentinels):
                    # skip their compute and scatter entirely.
                    cregs = nc.alloc_registers(
                        f"cnd{_rep}_{e}", bass.OrderedSet([
                            mybir.EngineType.PE, mybir.EngineType.DVE,
                            mybir.EngineType.Activation,
                            mybir.EngineType.Pool]))
                    for _r in cregs:
                        nc.reg_load(_r, nfs[e][:])
                    with tc.If(nc.snap(cregs) > 512 + C):
                        gu_chunk(512, C - 512)
                        down_tile(4)
                        nc.gpsimd.dma_scatter_add(y.ap(), ysb[:, 4:5, :],
                                                  ris[e][:, 0, 32:C // 16],
                                                  C - 512, C - 512, D)
                    # slots 0..511 always scatter; fires while the tail is
                    # still being computed
                    nc.gpsimd.dma_scatter_add(y.ap(), ysb[:, 0:4, :],
                                              ris[e][:, 0, 0:32],
                                              512, 512, D)

    nc.compile()
    return nc


_NC_CACHE = {}


def _get_nc():
    if "nc" not in _NC_CACHE:
        _NC_CACHE["nc"] = _build_nc()
    return _NC_CACHE["nc"]


def _in_maps_for(x, gate_w, w_gate, w_up, w_down):
    gate_w = np.asarray(gate_w, dtype=np.float32)
    xT = np.ascontiguousarray(x.T)
    xb = np.zeros((T + TPAD, D), dtype=BF16)
    xb[:T] = x.astype(BF16)
    tok1 = (np.arange(128, dtype=np.float32)[:, None]
            + 128.0 * np.arange(N_TT, dtype=np.float32)[None, :] + 1.0)
    tok1 = np.ascontiguousarray(tok1)
    ident = np.eye(128, dtype=np.float32)
    iota16 = np.broadcast_to((np.arange(128, dtype=np.int16) % 16)[:, None],
                             (128, 8)).copy()
    E_ = gate_w.shape[0]
    in_maps = []
    for c in range(N_CORES):
        e0 = c * N_LOC
        # top-8 selection and renorm are permutation-invariant; put this
        # core's experts in the first 8 gate columns.
        perm = list(range(e0, e0 + N_LOC)) + \
            [e for e in range(E_) if not (e0 <= e < e0 + N_LOC)]
        in_maps.append({
            "xT": xT,
            "xb": xb,
            "gwT": np.ascontiguousarray(gate_w[perm].T),
            "tok1": tok1,
            "ident": ident,
            "iota16": iota16,
            "wgT": np.ascontiguousarray(
                w_gate[e0:e0 + N_LOC].transpose(0, 2, 1)).astype(BF16),
            "wuT": np.ascontiguousarray(
                w_up[e0:e0 + N_LOC].transpose(0, 2, 1)).astype(BF16),
            "wdT": np.ascontiguousarray(
                w_down[e0:e0 + N_LOC].transpose(0, 2, 1)).astype(BF16),
        })
    return in_maps


def kernel(hidden_states, gate_w, w_gate, w_up, w_down):
    B, S, D_ = hidden_states.shape
    x = np.ascontiguousarray(np.asarray(hidden_states, dtype=np.float32)
                             .reshape(B * S, D_))
    nc = _get_nc()
    in_maps = _in_maps_for(x, gate_w, w_gate, w_up, w_down)
    res = bass_utils.run_bass_kernel_spmd(
        nc, in_maps, core_ids=list(range(N_CORES)))

    y = np.zeros((B * S, D_), np.float32)
    for c in range(N_CORES):
        y += np.asarray(res.results[c]["y"][:T], dtype=np.float32)
    return y.reshape(B, S, D_)


# revision 51
# speedup vs baseline: 1.2470x; 1.2470x over previous
"""Trainium2 Bass kernel for a 64-expert top-8 SwiGLU MoE layer.

Contract: kernel(**inputs) takes the FULL unsharded inputs
  hidden_states [2, 2048, 1024] f32, gate_w [64, 1024] f32,
  w_gate [64, 768, 1024] f32, w_up [64, 768, 1024] f32,
  w_down [64, 1024, 768] f32
and returns the full [2, 2048, 1024] f32 output.

Sharding: expert-parallel over 8 NeuronCores — core c owns experts
[8c, 8c+8). Every core computes the fp32 gate over all 4096 tokens
(top-8 of 64, renormalized), packs (token_id + weight) into one fp32
composite per (token, local expert), transposes those via the PE into
per-expert streams, compacts each stream with gpsimd sparse_gather
(sentinel-padded so the tail needs no fixup), then runs the SwiGLU
expert FFNs in bf16 (fp32 accumulation) on dma_gathered token rows and
scatter-adds the weighted bf16 rows into its partial output. The host
sums the 8 per-core partials.

gpsimd library hygiene: the only Pool ops are sparse_gather (lib 8)
and dma_gather/dma_scatter_add (mlp lib), grouped so exactly two
library loads occur, one of which hides under the gate phase.
"""

import sys

for _p in ("/opt/trn_rl_repo",):
    if _p not in sys.path:
        sys.path.insert(0, _p)

import numpy as np
import ml_dtypes

import concourse.bass as bass  # noqa: F401  (registers engine classes)
import concourse.bacc as bacc
import concourse.mybir as mybir
import concourse.tile as tile
from concourse import bass_utils

AF = mybir.ActivationFunctionType
ALU = mybir.AluOpType
DT = mybir.dt
BF16 = ml_dtypes.bfloat16

N_CORES = 8
N_LOC = 8          # experts per core
T = 4096           # tokens
TPAD = 16          # dummy rows for the sentinel token id T
D = 1024
F = 768
E = 64
CAP = 640          # per-expert token capacity (multiple of 128)
KD = D // 128      # 8
KF = F // 128      # 6
CT = CAP // 128    # 5
N_TT = T // 128    # 32 token tiles
SENT = float(T)    # sentinel composite value: token T, weight 0


def _build_nc(n_devices=N_CORES, repeat=1):
    C = CAP
    VE_F = (T + C) // 16       # 296 sparse_gather free size per expert
    n_chunks = [(0, 512), (512, C - 512)]
    d_chunks = [(0, 512), (512, 512)]

    nc = bacc.Bacc("TRN2", target_bir_lowering=False, debug=False,
                   num_devices=n_devices)

    xT = nc.dram_tensor("xT", [D, T], DT.float32, kind="ExternalInput")
    xb = nc.dram_tensor("xb", [T + TPAD, D], DT.bfloat16,
                        kind="ExternalInput")
    gwT = nc.dram_tensor("gwT", [D, E], DT.float32, kind="ExternalInput")
    tok1 = nc.dram_tensor("tok1", [128, N_TT], DT.float32,
                          kind="ExternalInput")
    ident = nc.dram_tensor("ident", [128, 128], DT.float32,
                           kind="ExternalInput")
    iota16 = nc.dram_tensor("iota16", [128, 8], DT.int16,
                            kind="ExternalInput")
    wgT = nc.dram_tensor("wgT", [N_LOC, D, F], DT.bfloat16,
                         kind="ExternalInput")
    wuT = nc.dram_tensor("wuT", [N_LOC, D, F], DT.bfloat16,
                         kind="ExternalInput")
    wdT = nc.dram_tensor("wdT", [N_LOC, F, D], DT.bfloat16,
                         kind="ExternalInput")
    y = nc.dram_tensor("y", [T + TPAD, D], DT.bfloat16,
                       kind="ExternalOutput")
    vt_dram = nc.dram_tensor("vt_scratch", [N_LOC, T], DT.float32,
                             kind="Internal")
    ti_dram = nc.dram_tensor("ti_scratch", [N_LOC, 16, 128], DT.int16,
                             kind="Internal")
    wf_dram = nc.dram_tensor("wf_scratch", [N_LOC, 16, CAP // 16],
                             DT.float32, kind="Internal")

    with tile.TileContext(nc) as tc:
      for _rep in range(repeat):  # >1 only for wall-clock benchmarking
        with (
            tc.tile_pool(name="gconst", bufs=1) as gconst,
            tc.tile_pool(name="vtp", bufs=1) as vtp,
            tc.tile_pool(name="route", bufs=1) as route,
            tc.tile_pool(name="decp", bufs=1) as decp,
            tc.tile_pool(name="wrp", bufs=1) as wrp,
        ):
            # ---- phase A: fp32 gate + composite pack + PE transpose ----
            with (
                tc.tile_pool(name="gx", bufs=2) as gx,
                tc.tile_pool(name="gps", bufs=3, space="PSUM") as gps,
                tc.tile_pool(name="tps", bufs=2, space="PSUM") as tps,
                tc.tile_pool(name="gtmp", bufs=3) as gtmp,
            ):
                # first (small) token chunk before the constant loads — SP
                # issues DMAs in order and the matmuls need tokens first;
                # a small chunk 0 cuts the PE startup wait
                chunk_lims = [0, 256, 1536, 2816, 4096]
                gw_sb = gconst.tile([128, KD, E], DT.float32)
                nc.sync.dma_start(gw_sb[:], gwT.ap().rearrange(
                    "(kc p) e -> p kc e", p=128))
                xt_ch = gx.tile([128, KD, 256], DT.float32, tag="xt")
                nc.sync.dma_start(
                    xt_ch[:], xT.ap()[:, 0:256].rearrange(
                        "(kc p) t -> p kc t", p=128))
                # small constants on the ACT queue; SP streams tokens
                id_sb = gconst.tile([128, 128], DT.float32)
                nc.scalar.dma_start(id_sb[:], ident.ap())
                tok1_sb = gconst.tile([128, N_TT], DT.float32)
                nc.scalar.dma_start(tok1_sb[:], tok1.ap())
                io16_sb = gconst.tile([128, 8], DT.int16)
                nc.scalar.dma_start(io16_sb[:], iota16.ap())

                # vT[e, t] = token0(t) + weight(t, e) if e in top8(t),
                # else -1, for the 8 local experts.
                vT = vtp.tile([N_LOC, T], DT.float32)
                # sparse_gather input, filled per gate chunk so only the
                # last sliver sits on the routing critical path
                ve_all = route.tile([16, N_LOC, VE_F], DT.float32)
                nc.vector.memset(ve_all[:, :, T // 16:], SENT)

                for ch in range(len(chunk_lims) - 1):
                    c0, c1 = chunk_lims[ch], chunk_lims[ch + 1]
                    if ch > 0:
                        xt_ch = gx.tile([128, KD, c1 - c0], DT.float32,
                                        tag="xt")
                        nc.sync.dma_start(
                            xt_ch[:],
                            xT.ap()[:, c0:c1].rearrange(
                                "(kc p) t -> p kc t", p=128))
                    for ti in range((c1 - c0) // 128):
                        tt = c0 // 128 + ti
                        psL = gps.tile([128, E], DT.float32, tag="psL")
                        for kc in range(KD):
                            nc.tensor.matmul(
                                psL[:],
                                xt_ch[:, kc, ti * 128:(ti + 1) * 128],
                                gw_sb[:, kc, :],
                                start=(kc == 0), stop=(kc == KD - 1),
                            )
                        mx8 = gtmp.tile([128, 8], DT.float32, tag="mx8")
                        nc.vector.max(mx8[:], psL[:])
                        negm = gtmp.tile([128, 1], DT.float32, tag="negm")
                        nc.vector.tensor_scalar_mul(negm[:], mx8[:, 0:1],
                                                    -1.0)
                        e8 = gtmp.tile([128, 8], DT.float32, tag="e8")
                        s8 = gtmp.tile([128, 1], DT.float32, tag="s8")
                        nc.scalar.activation(e8[:], mx8[:], AF.Exp,
                                             bias=negm[:], accum_out=s8[:])
                        rcp = gtmp.tile([128, 1], DT.float32, tag="rcp")
                        nc.vector.reciprocal(rcp[:], s8[:])
                        # local experts live in columns 0..8
                        ea8 = gtmp.tile([128, 8], DT.float32, tag="ea8")
                        nc.scalar.activation(ea8[:], psL[:, 0:N_LOC], AF.Exp,
                                             bias=negm[:])
                        ge8 = gtmp.tile([128, 8], DT.float32, tag="ge8")
                        nc.vector.tensor_scalar(ge8[:], psL[:, 0:N_LOC],
                                                mx8[:, 7:8], None,
                                                op0=ALU.is_ge)
                        wm8 = gtmp.tile([128, 8], DT.float32, tag="wm8")
                        nc.vector.scalar_tensor_tensor(
                            wm8[:], ea8[:], rcp[:], ge8[:],
                            op0=ALU.mult, op1=ALU.mult)
                        # composite: (wm8 + tok1)*ge - 1 =
                        #   tok0 + w (selected) / -1 (not selected)
                        vp = gtmp.tile([128, 8], DT.float32, tag="vp")
                        nc.vector.scalar_tensor_tensor(
                            vp[:], wm8[:], tok1_sb[:, tt:tt + 1], ge8[:],
                            op0=ALU.add, op1=ALU.mult)
                        vpack = gtmp.tile([128, 8], DT.float32, tag="vpack")
                        nc.vector.tensor_scalar_add(vpack[:], vp[:], -1.0)
                        psT = tps.tile([N_LOC, 128], DT.float32, tag="psT")
                        nc.tensor.transpose(psT[:], vpack[:], id_sb[:])
                        nc.vector.tensor_copy(
                            vT[:, tt * 128:(tt + 1) * 128], psT[:])
                    # stream vT to DRAM per chunk, then lift this chunk's
                    # tokens into the wrapped sparse_gather layout
                    nc.sync.dma_start(vt_dram.ap()[:, c0:c1],
                                      vT[:, c0:c1])
                    nc.sync.dma_start(
                        ve_all[:, :, c0 // 16:c1 // 16],
                        vt_dram.ap()[:, c0:c1].rearrange(
                            "e (p c) -> p e c", p=16))

            # ---- phase A2: per-expert compaction (sentinel-padded) ----
            # ve[e] = [16, 296]: head = vT row e partition-major, tail =
            # 640 sentinels. After compaction the first 640 outputs are
            # real-tokens-then-sentinels regardless of tail behaviour.
            toks = []
            nfs = []
            for e in range(N_LOC):
                tk = route.tile([16, VE_F], DT.float32, tag=f"tk{e}")
                nf = route.tile([1, 1], DT.uint32, tag=f"nf{e}")
                nc.gpsimd.sparse_gather(tk[:], ve_all[:, e, :],
                                        num_found=nf[:])
                toks.append(tk)
                nfs.append(nf)

            # decode composites: token ids (int16) and fractional gate
            # weights, bounced through DRAM so the index replication can be
            # a Pool dma_gather (row-replicate) and the weight unwrap an
            # arbitrary-rearrange DRAM read.
            ti_all = decp.tile([16, N_LOC, 128], DT.int16)
            tf_all = decp.tile([16, N_LOC, C // 16], DT.float32)
            wf_all = decp.tile([16, N_LOC, C // 16], DT.float32)
            wrow_all = wrp.tile([128, N_LOC, CT], DT.float32)
            nc.vector.memset(ti_all[:, :, C // 16:], 0)
            for e in range(N_LOC):
                tv = toks[e][:, 0:C // 16]
                # f32 -> i16 cast truncates toward zero = floor (tv >= 0)
                nc.vector.tensor_copy(ti_all[:, e, 0:C // 16], tv)
                nc.vector.tensor_copy(tf_all[:, e, :], ti_all[:, e, 0:C // 16])
                nc.vector.tensor_tensor(wf_all[:, e, :], tv, tf_all[:, e, :],
                                        op=ALU.subtract)
                nc.sync.dma_start(ti_dram.ap()[e], ti_all[:, e, :])
                nc.sync.dma_start(wf_dram.ap()[e], wf_all[:, e, :])
                # wrow[g*16+p, st] = wf[p, st*8+g]
                nc.sync.dma_start(
                    wrow_all[:, e, :],
                    wf_dram.ap()[e].rearrange("p (st g) -> g p st", g=8))
            wrows = [wrow_all[:, e, :] for e in range(N_LOC)]

            # ---- phase B: expert FFNs (bf16, fp32 accum) ----
            with (
                tc.tile_pool(name="xg", bufs=2) as xg,
                tc.tile_pool(name="rip", bufs=2) as rip,
                tc.tile_pool(name="wsb", bufs=2) as wsb,
                tc.tile_pool(name="hsb", bufs=2) as hsb,
                tc.tile_pool(name="ysb", bufs=2) as ysbp,
                tc.tile_pool(name="bps", bufs=2, space="PSUM") as bps,
                tc.tile_pool(name="dps", bufs=2, space="PSUM") as dps,
            ):
                ris = [None] * N_LOC
                XTs = [None] * N_LOC

                def prefetch(e):
                    # replicate idx row [16,128] -> [128,128] on Pool, then
                    # transpose-gather this expert's token rows
                    ri = rip.tile([128, 1, 128], DT.int16, tag="ri")
                    nc.gpsimd.dma_gather(ri[:], ti_dram.ap()[e],
                                         io16_sb[:], 128, 128, 128)
                    XT = xg.tile([128, KD, C], DT.bfloat16, tag="XT")
                    nc.gpsimd.dma_gather(XT[:], xb.ap(), ri[:, 0, 0:C // 16],
                                         C, C, D, transpose=True)
                    ris[e] = ri
                    XTs[e] = XT

                prefetch(0)
                for e in range(N_LOC):
                    XT = XTs[e]
                    # weight loads go on the ACT/DVE DGE queues so the
                    # routing-critical DMAs (vt/ve/decode) own the SP queue
                    wg_sb = wsb.tile([128, KD, F], DT.bfloat16, tag="wg")
                    nc.scalar.dma_start(wg_sb[:], wgT.ap()[e].rearrange(
                        "(kc p) f -> p kc f", p=128))
                    wu_sb = wsb.tile([128, KD, F], DT.bfloat16, tag="wu")
                    nc.scalar.dma_start(wu_sb[:], wuT.ap()[e].rearrange(
                        "(kc p) f -> p kc f", p=128))
                    wd_sb = wsb.tile([128, KF, D], DT.bfloat16, tag="wd")
                    nc.scalar.dma_start(wd_sb[:], wdT.ap()[e].rearrange(
                        "(kf p) d -> p kf d", p=128))

                    ht = hsb.tile([128, KF, C], DT.bfloat16, tag="ht")
                    ysb = ysbp.tile([128, CT, D], DT.bfloat16, tag="ysb")

                    def gu_chunk(c0, cn):
                        for ft in range(KF):
                            psg = bps.tile([128, 512], DT.float32,
                                           tag="psg")
                            psu = bps.tile([128, 512], DT.float32,
                                           tag="psu")
                            for kc in range(KD):
                                nc.tensor.matmul(
                                    psg[:, :cn],
                                    wg_sb[:, kc, ft * 128:(ft + 1) * 128],
                                    XT[:, kc, c0:c0 + cn],
                                    start=(kc == 0), stop=(kc == KD - 1))
                            for kc in range(KD):
                                nc.tensor.matmul(
                                    psu[:, :cn],
                                    wu_sb[:, kc, ft * 128:(ft + 1) * 128],
                                    XT[:, kc, c0:c0 + cn],
                                    start=(kc == 0), stop=(kc == KD - 1))
                            sg = hsb.tile([128, 512], DT.bfloat16, tag="sg")
                            nc.scalar.activation(sg[:, :cn], psg[:, :cn],
                                                 AF.Sigmoid)
                            t1 = hsb.tile([128, 512], DT.bfloat16, tag="t1")
                            nc.vector.tensor_tensor(
                                t1[:, :cn], sg[:, :cn],
                                psu[:, :cn], op=ALU.mult)
                            nc.vector.tensor_tensor(
                                ht[:, ft, c0:c0 + cn], t1[:, :cn],
                                psg[:, :cn], op=ALU.mult)

                    def down_tile(st):
                        for (d0, dn) in d_chunks:
                            psd = dps.tile([128, 512], DT.float32,
                                           tag="psd")
                            for kf in range(KF):
                                nc.tensor.matmul(
                                    psd[:, :dn],
                                    ht[:, kf, st * 128:(st + 1) * 128],
                                    wd_sb[:, kf, d0:d0 + dn],
                                    start=(kf == 0), stop=(kf == KF - 1))
                            nc.vector.tensor_scalar(
                                ysb[:, st, d0:d0 + dn], psd[:, :dn],
                                wrows[e][:, st:st + 1], None,
                                op0=ALU.mult)

                    gu_chunk(0, 512)
                    for st in range(4):
                        down_tile(st)

                    if e + 1 < N_LOC:
                        prefetch(e + 1)

                    # slots 512..639 are all sentinels when this expert got
                    # <= 512 tokens (num_found = real + 640 sentinels):
                    # skip their compute and scatter entirely.
                    cregs = nc.alloc_registers(
                        f"cnd{_rep}_{e}", bass.OrderedSet([
                            mybir.EngineType.PE, mybir.EngineType.DVE,
                            mybir.EngineType.Activation,
                            mybir.EngineType.Pool]))
                    for _r in cregs:
                        nc.reg_load(_r, nfs[e][:])
                    with tc.If(nc.snap(cregs) > 512 + C):
                        gu_chunk(512, C - 512)
                        down_tile(4)
                        nc.gpsimd.dma_scatter_add(y.ap(), ysb[:, 4:5, :],
                                                  ris[e][:, 0, 32:C // 16],
                                                  C - 512, C - 512, D)
                    # slots 0..511 always scatter; fires while the tail is
                    # still being computed
                    nc.gpsimd.dma_scatter_add(y.ap(), ysb[:, 0:4, :],
                                              ris[e][:, 0, 0:32],
                                              512, 512, D)

    nc.compile()
    return nc


_NC_CACHE = {}


def _get_nc():
    if "nc" not in _NC_CACHE:
        _NC_CACHE["nc"] = _build_nc()
    return _NC_CACHE["nc"]


def _in_maps_for(x, gate_w, w_gate, w_up, w_down):
    gate_w = np.asarray(gate_w, dtype=np.float32)
    xT = np.ascontiguousarray(x.T)
    xb = np.zeros((T + TPAD, D), dtype=BF16)
    xb[:T] = x.astype(BF16)
    tok1 = (np.arange(128, dtype=np.float32)[:, None]
            + 128.0 * np.arange(N_TT, dtype=np.float32)[None, :] + 1.0)
    tok1 = np.ascontiguousarray(tok1)
    ident = np.eye(128, dtype=np.float32)
    iota16 = np.broadcast_to((np.arange(128, dtype=np.int16) % 16)[:, None],
                             (128, 8)).copy()
    E_ = gate_w.shape[0]
    in_maps = []
    for c in range(N_CORES):
        e0 = c * N_LOC
        # top-8 selection and renorm are permutation-invariant; put this
        # core's experts in the first 8 gate columns.
        perm = list(range(e0, e0 + N_LOC)) + \
            [e for e in range(E_) if not (e0 <= e < e0 + N_LOC)]
        in_maps.append({
            "xT": xT,
            "xb": xb,
            "gwT": np.ascontiguousarray(gate_w[perm].T),
            "tok1": tok1,
            "ident": ident,
            "iota16": iota16,
            "wgT": np.ascontiguousarray(
                w_gate[e0:e0 + N_LOC].transpose(0, 2, 1)).astype(BF16),
            "wuT": np.ascontiguousarray(
                w_up[e0:e0 + N_LOC].transpose(0, 2, 1)).astype(BF16),
            "wdT": np.ascontiguousarray(
                w_down[e0:e0 + N_LOC].transpose(0, 2, 1)).astype(BF16),
        })
    return in_maps


def kernel(hidden_states, gate_w, w_gate, w_up, w_down):
    B, S, D_ = hidden_states.shape
    x = np.ascontiguousarray(np.asarray(hidden_states, dtype=np.float32)
                             .reshape(B * S, D_))
    nc = _get_nc()
    in_maps = _in_maps_for(x, gate_w, w_gate, w_up, w_down)
    res = bass_utils.run_bass_kernel_spmd(
        nc, in_maps, core_ids=list(range(N_CORES)))

    y = np.zeros((B * S, D_), np.float32)
    for c in range(N_CORES):
        y += np.asarray(res.results[c]["y"][:T], dtype=np.float32)
    return y.reshape(B, S, D_)
